# revision 11
# baseline (speedup 1.0000x reference)
"""Int8 Llama MLP (W8A8) on 8 Trainium2 NeuronCores.

Strategy: tensor-parallel over the intermediate dim I (11008 -> 1376/core).
Each core computes, for all 8192 tokens:
    gT  = gate_wT_shard-contraction vs quantized-x  (PSUM f32, exact int math in bf16)
    uT  = same for up
    hT  = quantize(silu_f16(gT*gs) * (uT*us) / ds_in)  as bf16 integers
    out_partial[T, H] bf16 = hT.T-contraction vs down_wT_shard, scaled
Host: quantize+transpose+tile inputs (exact int8 -> bf16), sum the 8 partial
outputs in f32.

Key PE-efficiency detail (HW-measured): a matmul whose stationary operand
differs from the previous one pays an un-hidden ~260ns LDWEIGHTS; giving each
stationary tile >=2 consecutive matmuls (different PSUM banks) hides almost
all of it (467 -> 224 ns/MM at N=512).  Hence:
  - token blocks of 1024 = 2 x 512-wide MMs per gate/up weight tile
  - down proj runs transposed (stationary = h-slice [128i x 128tok], moving =
    down_wT [128i x 512h]): 4 MMs per stationary across a 2048-wide H half,
    output directly in [T, H] layout (no host transpose), down_wT resident.

All matmuls run with int-valued bf16 operands: products are exact in the f32
PSUM accumulation; only >2^24 running-sum rounding and the bf16 partial-out
rounding (~1e-3 rel) differ from the int32 reference.
"""

import numpy as np
import ml_dtypes

import concourse.bass as bass
import concourse.mybir as mybir
import concourse.tile as tile
from concourse import bacc
from concourse.bass_utils import run_bass_kernel_spmd

T, H, I = 8192, 4096, 11008
NCORES = 8
IP = 11264                 # I zero-padded to a multiple of 8*128
ISH = IP // NCORES         # 1408 intermediate rows per core
NI = ISH // 128            # 11 partition tiles of the I-shard
KO = H // 128              # 32 k-chunks for gate/up contraction
TB = 1024                  # token block
NB = T // TB               # 8 token blocks
NT = TB // 128             # 8 token sub-tiles per block (down stationary)
HG = 512                   # H chunk per down matmul (moving free dim)
NHP = 4                    # H passes in down phase
HPW = H // NHP             # 1024 H columns per pass
NHG = HPW // HG            # 2 moving chunks per pass = 2 MMs per stationary

F32 = mybir.dt.float32
F16 = mybir.dt.float16
BF16 = mybir.dt.bfloat16
MAGIC = 12582912.0         # 1.5 * 2^23: float32 round-to-nearest-even trick

_prog_cache = {}


def _build_program(share_x: bool, gsc: float, usc_over_dis: float, dsc: float):
    key = (share_x, gsc, usc_over_dis, dsc)
    if key in _prog_cache:
        return _prog_cache[key]

    nc = bacc.Bacc(None)
    xq = nc.declare_dram_parameter("xq", [NB, 128, KO, TB], BF16, isOutput=False)
    if share_x:
        xq2 = xq
    else:
        xq2 = nc.declare_dram_parameter("xq2", [NB, 128, KO, TB], BF16, isOutput=False)
    wg = nc.declare_dram_parameter("wg", [NI, 128, KO, 128], BF16, isOutput=False)
    wu = nc.declare_dram_parameter("wu", [NI, 128, KO, 128], BF16, isOutput=False)
    wdt = nc.declare_dram_parameter("wdt", [NI, 128, H], BF16, isOutput=False)
    out = nc.declare_dram_parameter("out", [T, H], BF16, isOutput=True)

    ACT = mybir.ActivationFunctionType
    ALU = mybir.AluOpType

    with tile.TileContext(nc) as tc:
        with (
            tc.tile_pool(name="pwd", bufs=1) as pwd,
            tc.tile_pool(name="px", bufs=1) as px,
            tc.tile_pool(name="pw", bufs=2) as pw,
            tc.tile_pool(name="pht", bufs=NI) as pht,
            tc.tile_pool(name="ptmp", bufs=1) as ptmp,
            tc.tile_pool(name="pout", bufs=3) as pout,
            tc.tile_pool(name="psg", bufs=1, space="PSUM") as psg,
            tc.tile_pool(name="psu", bufs=1, space="PSUM") as psu,
            tc.tile_pool(name="psd", bufs=2, space="PSUM") as psd,
        ):
            qs = [nc.gpsimd, nc.scalar, nc.sync]

            # x is single-buffered: block b+1's DMA is emitted right after the
            # last x read of block b (end of the gate/up phase) on the
            # otherwise-idle gpsimd queue, so it lands during b's down phase.
            # x DMA in 1MB chunks round-robined over the queues so the
            # small down-phase output DMAs can interleave (the DMA engines
            # are a shared resource).
            def load_x(b):
                x_sb = px.tile([128, KO, TB], BF16, tag="x", name="x_sb")
                for c in range(8):
                    qs[c % 3].dma_start(x_sb[:, c * 4:(c + 1) * 4, :],
                                        xq[b, :, c * 4:(c + 1) * 4, :])
                if share_x:
                    x2_sb = x_sb
                else:
                    x2_sb = px.tile([128, KO, TB], BF16, tag="x2", name="x2_sb")
                    for c in range(8):
                        qs[c % 3].dma_start(x2_sb[:, c * 4:(c + 1) * 4, :],
                                            xq2[b, :, c * 4:(c + 1) * 4, :])
                return x_sb, x2_sb

            x_cur = load_x(0)
            # down_wT load comes after x(0): it is not needed until the first
            # down phase ~300us in, while x(0) gates the very first matmul
            wdt_sb = pwd.tile([128, NI, H], BF16)
            for k in range(NI):
                qs[k % 2].dma_start(wdt_sb[:, k, :], wdt[k])
            for b in range(NB):
                x_sb, x2_sb = x_cur

                ht_tiles = []
                for i in range(NI):
                    wg_sb = pw.tile([128, KO, 128], BF16, tag="w", name="wg_sb")
                    nc.sync.dma_start(wg_sb[:], wg[i])
                    wu_sb = pw.tile([128, KO, 128], BF16, tag="w", name="wu_sb")
                    nc.sync.dma_start(wu_sb[:], wu[i])

                    # gate / up chains: both 512-halves share each weight tile
                    g_ps = [psg.tile([128, 512], F32, tag=f"g{h}", name=f"g{h}")
                            for h in range(2)]
                    for ko in range(KO):
                        wt = wg_sb[:, ko, :]
                        for h in range(2):
                            nc.tensor.matmul(g_ps[h][:], wt,
                                             x_sb[:, ko, h * 512:(h + 1) * 512],
                                             start=(ko == 0), stop=(ko == KO - 1))
                    u_ps = [psu.tile([128, 512], F32, tag=f"u{h}", name=f"u{h}")
                            for h in range(2)]
                    for ko in range(KO):
                        wt = wu_sb[:, ko, :]
                        for h in range(2):
                            nc.tensor.matmul(u_ps[h][:], wt,
                                             x2_sb[:, ko, h * 512:(h + 1) * 512],
                                             start=(ko == 0), stop=(ko == KO - 1))

                    # hidden = silu(f16(g*gsc)) * (u*usc/dis), round+clip to int8
                    ht_i = pht.tile([128, TB], BF16, tag="ht", name="ht_i")
                    for h in range(2):
                        sl = slice(h * 512, (h + 1) * 512)
                        t16 = ptmp.tile([128, 512], F16, tag=f"t16{h}", name="t16")
                        nc.scalar.activation(t16[:], g_ps[h][:], ACT.Copy, scale=gsc)
                        s16 = ptmp.tile([128, 512], F16, tag=f"s16{h}", name="s16")
                        nc.scalar.activation(s16[:], t16[:], ACT.Sigmoid)
                        sl16 = ptmp.tile([128, 512], F16, tag=f"sl16{h}", name="sl16")
                        nc.vector.tensor_tensor(sl16[:], t16[:], s16[:], ALU.mult)
                        h32 = ptmp.tile([128, 512], F32, tag=f"h32{h}", name="h32")
                        nc.vector.scalar_tensor_tensor(h32[:], u_ps[h][:],
                                                       usc_over_dis,
                                                       sl16[:], ALU.mult, ALU.mult)
                        # clamp to (-128.49, 127.49) pre-round: keeps magic-add
                        # in exact range and matches round-then-clip on boundaries
                        c32 = ptmp.tile([128, 512], F32, tag=f"c32{h}", name="c32")
                        nc.vector.tensor_scalar(c32[:], h32[:], -128.49, 127.49,
                                                ALU.max, ALU.min)
                        nc.vector.tensor_scalar(ht_i[:, sl], c32[:], MAGIC, MAGIC,
                                                ALU.add, ALU.subtract)
                    ht_tiles.append(ht_i)

                if b + 1 < NB:
                    x_cur = load_x(b + 1)

                # down proj, transposed: stationary = ht[k][:, 128-token slice],
                # moving = down_wT [128, 512 of H]; the moving chunks per pass
                # share each stationary -> LDWEIGHTS amortized.
                for p in range(NHP):
                    for t in range(NT):
                        tsl = slice(t * 128, (t + 1) * 128)
                        d_ps = [psd.tile([128, HG], F32, tag=f"d{g}", name=f"d{g}")
                                for g in range(NHG)]
                        for k in range(NI):
                            st = ht_tiles[k][:, tsl]
                            for g in range(NHG):
                                csl = slice(p * HPW + g * HG, p * HPW + (g + 1) * HG)
                                nc.tensor.matmul(d_ps[g][:], st, wdt_sb[:, k, csl],
                                                 start=(k == 0), stop=(k == NI - 1))
                        for g in range(NHG):
                            o_sb = pout.tile([128, HG], BF16, tag="o", name="o_sb")
                            nc.scalar.activation(o_sb[:], d_ps[g][:], ACT.Copy,
                                                 scale=dsc)
                            nc.sync.dma_start(
                                out[b * TB + t * 128:b * TB + (t + 1) * 128,
                                    p * HPW + g * HG:p * HPW + (g + 1) * HG],
                                o_sb[:])

    nc.finalize()
    _prog_cache[key] = nc
    return nc


def _quant_tile_x(x: np.ndarray, scale: float) -> np.ndarray:
    """clip(round(x/scale)) -> tiled [NB, 128, KO, TB] bf16 (exact ints)."""
    q = np.clip(np.round(x / np.float32(scale)), -128, 127).astype(np.float32)
    return np.ascontiguousarray(
        q.reshape(NB, TB, KO, 128).transpose(0, 3, 2, 1)
    ).astype(ml_dtypes.bfloat16)


def _prepare_in_maps(x, gate_w, up_w, down_w, gis, uis, share_x):
    xq = _quant_tile_x(np.asarray(x, np.float32), gis)
    xq2 = None if share_x else _quant_tile_x(np.asarray(x, np.float32), uis)

    # zero-pad I (11008 -> 11264): padded gate/up rows give hidden=0 and the
    # padded down rows are 0, so the result is unchanged
    gw = np.zeros((IP, H), np.int8); gw[:I] = np.asarray(gate_w)
    uw = np.zeros((IP, H), np.int8); uw[:I] = np.asarray(up_w)
    dwt = np.zeros((IP, H), np.int8); dwt[:I] = np.asarray(down_w).T

    in_maps = []
    for c in range(NCORES):
        i0, i1 = c * ISH, (c + 1) * ISH
        wg_c = np.ascontiguousarray(
            gw[i0:i1].reshape(NI, 128, KO, 128).transpose(0, 3, 2, 1)
        ).astype(ml_dtypes.bfloat16)
        wu_c = np.ascontiguousarray(
            uw[i0:i1].reshape(NI, 128, KO, 128).transpose(0, 3, 2, 1)
        ).astype(ml_dtypes.bfloat16)
        wdt_c = np.ascontiguousarray(
            dwt[i0:i1].reshape(NI, 128, H)
        ).astype(ml_dtypes.bfloat16)
        m = {"xq": xq, "wg": wg_c, "wu": wu_c, "wdt": wdt_c}
        if not share_x:
            m["xq2"] = xq2
        in_maps.append(m)
    return in_maps


def kernel(x, gate_w, up_w, down_w,
           gate_in_scale, gate_w_scale,
           up_in_scale, up_w_scale,
           down_in_scale, down_w_scale):
    gis = float(gate_in_scale)
    uis = float(up_in_scale)
    dis = float(down_in_scale)
    gsc = float(np.float32(gis) * np.float32(gate_w_scale))
    usc = float(np.float32(uis) * np.float32(up_w_scale))
    dsc = float(np.float32(dis) * np.float32(down_w_scale))
    share_x = (np.float32(gis) == np.float32(uis))

    nc = _build_program(share_x, gsc, usc / dis, dsc)
    in_maps = _prepare_in_maps(x, gate_w, up_w, down_w, gis, uis, share_x)

    res = run_bass_kernel_spmd(nc, in_maps, list(range(NCORES)))

    acc = np.zeros((T, H), np.float32)
    for c in range(NCORES):
        acc += res.results[c]["out"].astype(np.float32)
    return acc


# revision 16
# speedup vs baseline: 1.5927x; 1.5927x over previous
"""Int8 Llama MLP (W8A8) on 8 Trainium2 NeuronCores.

Strategy: tensor-parallel over the intermediate dim I (11008 -> 1376/core).
Each core computes, for all 8192 tokens:
    gT  = gate_wT_shard-contraction vs quantized-x  (PSUM f32, exact int math in bf16)
    uT  = same for up
    hT  = quantize(silu_f16(gT*gs) * (uT*us) / ds_in)  as bf16 integers
    out_partial[T, H] bf16 = hT.T-contraction vs down_wT_shard, scaled
Host: quantize+transpose+tile inputs (exact int8 -> bf16), sum the 8 partial
outputs in f32.

Key PE-efficiency detail (HW-measured): a matmul whose stationary operand
differs from the previous one pays an un-hidden ~260ns LDWEIGHTS; giving each
stationary tile >=2 consecutive matmuls (different PSUM banks) hides almost
all of it (467 -> 224 ns/MM at N=512).  Hence:
  - token blocks of 1024 = 2 x 512-wide MMs per gate/up weight tile
  - down proj runs transposed (stationary = h-slice [128i x 128tok], moving =
    down_wT [128i x 512h]): 4 MMs per stationary across a 2048-wide H half,
    output directly in [T, H] layout (no host transpose), down_wT resident.

All matmuls run with int-valued bf16 operands: products are exact in the f32
PSUM accumulation; only >2^24 running-sum rounding and the bf16 partial-out
rounding (~1e-3 rel) differ from the int32 reference.
"""

import numpy as np
import ml_dtypes

import concourse.bass as bass
import concourse.mybir as mybir
import concourse.tile as tile
from concourse import bacc
from concourse.bass_utils import run_bass_kernel_spmd

T, H, I = 8192, 4096, 11008
NCORES = 8
IP = 11264                 # I zero-padded to a multiple of 8*128
ISH = IP // NCORES         # 1408 intermediate rows per core
NI = ISH // 128            # 11 partition tiles of the I-shard
KO = H // 128              # 32 k-chunks for gate/up contraction
TB = 1024                  # token block
NB = T // TB               # 8 token blocks
NT = TB // 128             # 8 token sub-tiles per block (down stationary)
HG = 512                   # H chunk per down matmul (moving free dim)
NHP = 4                    # H passes in down phase
HPW = H // NHP             # 1024 H columns per pass
NHG = HPW // HG            # 2 moving chunks per pass = 2 MMs per stationary

F32 = mybir.dt.float32
F16 = mybir.dt.float16
BF16 = mybir.dt.bfloat16
MAGIC = 12582912.0         # 1.5 * 2^23: float32 round-to-nearest-even trick

_prog_cache = {}


def _build_program(share_x: bool, gsc: float, usc_over_dis: float, dsc: float):
    key = (share_x, gsc, usc_over_dis, dsc)
    if key in _prog_cache:
        return _prog_cache[key]

    nc = bacc.Bacc(None)
    xq = nc.declare_dram_parameter("xq", [NB, 128, KO, TB], BF16, isOutput=False)
    if share_x:
        xq2 = xq
    else:
        xq2 = nc.declare_dram_parameter("xq2", [NB, 128, KO, TB], BF16, isOutput=False)
    wg = nc.declare_dram_parameter("wg", [NI, 128, KO, 128], BF16, isOutput=False)
    wu = nc.declare_dram_parameter("wu", [NI, 128, KO, 128], BF16, isOutput=False)
    wdt = nc.declare_dram_parameter("wdt", [NI, 128, H], BF16, isOutput=False)
    out = nc.declare_dram_parameter("out", [T, H], BF16, isOutput=True)

    ACT = mybir.ActivationFunctionType
    ALU = mybir.AluOpType

    with tile.TileContext(nc) as tc:
        with (
            tc.tile_pool(name="pwd", bufs=1) as pwd,
            tc.tile_pool(name="px", bufs=1) as px,
            tc.tile_pool(name="pw", bufs=2) as pw,
            tc.tile_pool(name="pht", bufs=NI) as pht,
            tc.tile_pool(name="ptmp", bufs=1) as ptmp,
            tc.tile_pool(name="pout", bufs=3) as pout,
            tc.tile_pool(name="psg", bufs=1, space="PSUM") as psg,
            tc.tile_pool(name="psu", bufs=1, space="PSUM") as psu,
            tc.tile_pool(name="psd", bufs=2, space="PSUM") as psd,
        ):
            qs = [nc.gpsimd, nc.scalar, nc.sync]

            # x is single-buffered: block b+1's DMA is emitted right after the
            # last x read of block b (end of the gate/up phase) on the
            # otherwise-idle gpsimd queue, so it lands during b's down phase.
            # x DMA in 1MB chunks round-robined over the queues so the
            # small down-phase output DMAs can interleave (the DMA engines
            # are a shared resource).
            def load_x(b):
                x_sb = px.tile([128, KO, TB], BF16, tag="x", name="x_sb")
                for c in range(8):
                    qs[c % 3].dma_start(x_sb[:, c * 4:(c + 1) * 4, :],
                                        xq[b, :, c * 4:(c + 1) * 4, :])
                if share_x:
                    x2_sb = x_sb
                else:
                    x2_sb = px.tile([128, KO, TB], BF16, tag="x2", name="x2_sb")
                    for c in range(8):
                        qs[c % 3].dma_start(x2_sb[:, c * 4:(c + 1) * 4, :],
                                            xq2[b, :, c * 4:(c + 1) * 4, :])
                return x_sb, x2_sb

            x_cur = load_x(0)
            # down_wT load comes after x(0): it is not needed until the first
            # down phase ~300us in, while x(0) gates the very first matmul
            wdt_sb = pwd.tile([128, NI, H], BF16)
            for k in range(NI):
                qs[k % 2].dma_start(wdt_sb[:, k, :], wdt[k])
            for b in range(NB):
                x_sb, x2_sb = x_cur

                ht_tiles = []
                for i in range(NI):
                    wg_sb = pw.tile([128, KO, 128], BF16, tag="w", name="wg_sb")
                    nc.sync.dma_start(wg_sb[:], wg[i])
                    wu_sb = pw.tile([128, KO, 128], BF16, tag="w", name="wu_sb")
                    nc.sync.dma_start(wu_sb[:], wu[i])

                    # gate / up chains: both 512-halves share each weight tile
                    g_ps = [psg.tile([128, 512], F32, tag=f"g{h}", name=f"g{h}")
                            for h in range(2)]
                    for ko in range(KO):
                        wt = wg_sb[:, ko, :]
                        for h in range(2):
                            nc.tensor.matmul(g_ps[h][:], wt,
                                             x_sb[:, ko, h * 512:(h + 1) * 512],
                                             start=(ko == 0), stop=(ko == KO - 1))
                    u_ps = [psu.tile([128, 512], F32, tag=f"u{h}", name=f"u{h}")
                            for h in range(2)]
                    for ko in range(KO):
                        wt = wu_sb[:, ko, :]
                        for h in range(2):
                            nc.tensor.matmul(u_ps[h][:], wt,
                                             x2_sb[:, ko, h * 512:(h + 1) * 512],
                                             start=(ko == 0), stop=(ko == KO - 1))

                    # hidden = silu(f16(g*gsc)) * (u*usc/dis), round+clip to int8
                    ht_i = pht.tile([128, TB], BF16, tag="ht", name="ht_i")
                    for h in range(2):
                        sl = slice(h * 512, (h + 1) * 512)
                        t16 = ptmp.tile([128, 512], F16, tag=f"t16{h}", name="t16")
                        nc.scalar.activation(t16[:], g_ps[h][:], ACT.Copy, scale=gsc)
                        s16 = ptmp.tile([128, 512], F16, tag=f"s16{h}", name="s16")
                        nc.scalar.activation(s16[:], t16[:], ACT.Sigmoid)
                        sl16 = ptmp.tile([128, 512], F16, tag=f"sl16{h}", name="sl16")
                        nc.vector.tensor_tensor(sl16[:], t16[:], s16[:], ALU.mult)
                        h32 = ptmp.tile([128, 512], F32, tag=f"h32{h}", name="h32")
                        nc.vector.scalar_tensor_tensor(h32[:], u_ps[h][:],
                                                       usc_over_dis,
                                                       sl16[:], ALU.mult, ALU.mult)
                        # clamp to (-128.49, 127.49) pre-round: keeps magic-add
                        # in exact range and matches round-then-clip on boundaries
                        c32 = ptmp.tile([128, 512], F32, tag=f"c32{h}", name="c32")
                        nc.vector.tensor_scalar(c32[:], h32[:], -128.49, 127.49,
                                                ALU.max, ALU.min)
                        nc.vector.tensor_scalar(ht_i[:, sl], c32[:], MAGIC, MAGIC,
                                                ALU.add, ALU.subtract)
                    ht_tiles.append(ht_i)

                if b + 1 < NB:
                    x_cur = load_x(b + 1)

                # down proj, transposed: stationary = ht[k][:, 128-token slice],
                # moving = down_wT [128, 512 of H]; the moving chunks per pass
                # share each stationary -> LDWEIGHTS amortized.
                for p in range(NHP):
                    for t in range(NT):
                        tsl = slice(t * 128, (t + 1) * 128)
                        d_ps = [psd.tile([128, HG], F32, tag=f"d{g}", name=f"d{g}")
                                for g in range(NHG)]
                        for k in range(NI):
                            st = ht_tiles[k][:, tsl]
                            for g in range(NHG):
                                csl = slice(p * HPW + g * HG, p * HPW + (g + 1) * HG)
                                nc.tensor.matmul(d_ps[g][:], st, wdt_sb[:, k, csl],
                                                 start=(k == 0), stop=(k == NI - 1))
                        for g in range(NHG):
                            o_sb = pout.tile([128, HG], BF16, tag="o", name="o_sb")
                            nc.scalar.activation(o_sb[:], d_ps[g][:], ACT.Copy,
                                                 scale=dsc)
                            nc.sync.dma_start(
                                out[b * TB + t * 128:b * TB + (t + 1) * 128,
                                    p * HPW + g * HG:p * HPW + (g + 1) * HG],
                                o_sb[:])

    nc.finalize()
    _prog_cache[key] = nc
    return nc


def _quant_tile_x(x: np.ndarray, scale: float) -> np.ndarray:
    """clip(round(x/scale)) -> tiled [NB, 128, KO, TB] bf16 (exact ints)."""
    q = np.clip(np.round(x / np.float32(scale)), -128, 127).astype(np.float32)
    return np.ascontiguousarray(
        q.reshape(NB, TB, KO, 128).transpose(0, 3, 2, 1)
    ).astype(ml_dtypes.bfloat16)


def _prepare_in_maps(x, gate_w, up_w, down_w, gis, uis, share_x):
    xq = _quant_tile_x(np.asarray(x, np.float32), gis)
    xq2 = None if share_x else _quant_tile_x(np.asarray(x, np.float32), uis)

    # zero-pad I (11008 -> 11264): padded gate/up rows give hidden=0 and the
    # padded down rows are 0, so the result is unchanged
    gw = np.zeros((IP, H), np.int8); gw[:I] = np.asarray(gate_w)
    uw = np.zeros((IP, H), np.int8); uw[:I] = np.asarray(up_w)
    dwt = np.zeros((IP, H), np.int8); dwt[:I] = np.asarray(down_w).T

    in_maps = []
    for c in range(NCORES):
        i0, i1 = c * ISH, (c + 1) * ISH
        wg_c = np.ascontiguousarray(
            gw[i0:i1].reshape(NI, 128, KO, 128).transpose(0, 3, 2, 1)
        ).astype(ml_dtypes.bfloat16)
        wu_c = np.ascontiguousarray(
            uw[i0:i1].reshape(NI, 128, KO, 128).transpose(0, 3, 2, 1)
        ).astype(ml_dtypes.bfloat16)
        wdt_c = np.ascontiguousarray(
            dwt[i0:i1].reshape(NI, 128, H)
        ).astype(ml_dtypes.bfloat16)
        m = {"xq": xq, "wg": wg_c, "wu": wu_c, "wdt": wdt_c}
        if not share_x:
            m["xq2"] = xq2
        in_maps.append(m)
    return in_maps


def kernel(x, gate_w, up_w, down_w,
           gate_in_scale, gate_w_scale,
           up_in_scale, up_w_scale,
           down_in_scale, down_w_scale):
    gis = float(gate_in_scale)
    uis = float(up_in_scale)
    dis = float(down_in_scale)
    gsc = float(np.float32(gis) * np.float32(gate_w_scale))
    usc = float(np.float32(uis) * np.float32(up_w_scale))
    dsc = float(np.float32(dis) * np.float32(down_w_scale))
    share_x = (np.float32(gis) == np.float32(uis))

    nc = _build_program(share_x, gsc, usc / dis, dsc)
    in_maps = _prepare_in_maps(x, gate_w, up_w, down_w, gis, uis, share_x)

    res = run_bass_kernel_spmd(nc, in_maps, list(range(NCORES)))

    acc = np.zeros((T, H), np.float32)
    for c in range(NCORES):
        acc += res.results[c]["out"].astype(np.float32)
    return acc


# revision 17
# speedup vs baseline: 1.7506x; 1.0992x over previous
"""Int8 Llama MLP (W8A8) on 8 Trainium2 NeuronCores.

Strategy: tensor-parallel over the intermediate dim I (11008 -> 1376/core).
Each core computes, for all 8192 tokens:
    gT  = gate_wT_shard-contraction vs quantized-x  (PSUM f32, exact int math in bf16)
    uT  = same for up
    hT  = quantize(silu_f16(gT*gs) * (uT*us) / ds_in)  as bf16 integers
    partial[T, H] bf16 = hT.T-contraction vs down_wT_shard, scaled
Per token block, a ReduceScatter(add) over the 8 cores sums the partials
on-device; each core returns its 128-token slice of every block.  The host
only reassembles slices (no reduction, no transpose).

Perf notes (HW-measured on this setup):
  - per-execution runtime cost scales with operand bytes, so weights ship as
    int8 and are dequantized to bf16 on the idle DVE; the output is 8.4MB
    per core instead of 134MB of f32 partials.
  - a matmul whose stationary operand differs from the previous one can pay
    un-hidden LDWEIGHTS; every stationary tile here feeds >=2 consecutive
    matmuls (token blocks of 1024 = 2 x 512-wide MMs per gate/up weight;
    the transposed down proj reuses each h-slice across H chunks).
  - the down proj runs transposed (stationary = h-slice [128i x 128tok],
    moving = down_wT [128i x 512h]) so the output lands in [T, H] layout,
    which is what ReduceScatter over tokens needs.

All matmuls run with int-valued bf16 operands: products are exact in the f32
PSUM accumulation; only >2^24 running-sum rounding and the bf16 partial/
collective rounding (~1e-3 rel) differ from the int32 reference.
"""

import numpy as np
import ml_dtypes

import concourse.bass as bass
import concourse.mybir as mybir
import concourse.tile as tile
from concourse import bacc
from concourse.bass_utils import run_bass_kernel_spmd

T, H, I = 8192, 4096, 11008
NCORES = 8
IP = 11264                 # I zero-padded to a multiple of 8*128
ISH = IP // NCORES         # 1408 intermediate rows per core
NI = ISH // 128            # 11 partition tiles of the I-shard
KO = H // 128              # 32 k-chunks for gate/up contraction
TB = 1024                  # token block
NB = T // TB               # 8 token blocks
NT = TB // 128             # 8 token sub-tiles per block (down stationary)
HG = 512                   # H chunk per down matmul (moving free dim)
NHP = 4                    # H passes in down phase
HPW = H // NHP             # 1024 H columns per pass
NHG = HPW // HG            # 2 moving chunks per pass = 2 MMs per stationary

F32 = mybir.dt.float32
F16 = mybir.dt.float16
BF16 = mybir.dt.bfloat16
I8 = mybir.dt.int8
MAGIC = 12582912.0         # 1.5 * 2^23: float32 round-to-nearest-even trick

_prog_cache = {}


def _build_program(share_x: bool, gsc: float, usc_over_dis: float, dsc: float):
    key = (share_x, gsc, usc_over_dis, dsc)
    if key in _prog_cache:
        return _prog_cache[key]

    nc = bacc.Bacc(None)
    xq = nc.declare_dram_parameter("xq", [NB, 128, KO, TB], BF16, isOutput=False)
    if share_x:
        xq2 = xq
    else:
        xq2 = nc.declare_dram_parameter("xq2", [NB, 128, KO, TB], BF16, isOutput=False)
    wg8 = nc.declare_dram_parameter("wg8", [NI, 128, KO, 128], I8, isOutput=False)
    wu8 = nc.declare_dram_parameter("wu8", [NI, 128, KO, 128], I8, isOutput=False)
    wdt8 = nc.declare_dram_parameter("wdt8", [NI, 128, H], I8, isOutput=False)
    out = nc.declare_dram_parameter("out", [NB, 128, H], BF16, isOutput=True)
    rs_in = nc.dram_tensor("rs_in", [NB, TB, H], BF16)
    rs_out = nc.dram_tensor("rs_out", [NB, 128, H], BF16)

    ACT = mybir.ActivationFunctionType
    ALU = mybir.AluOpType
    from concourse.tile_rust import add_dep_helper

    with tile.TileContext(nc) as tc:
        with (
            tc.tile_pool(name="px", bufs=1) as px,
            tc.tile_pool(name="pwi", bufs=2) as pwi,
            tc.tile_pool(name="pw", bufs=2) as pw,
            tc.tile_pool(name="pdw8", bufs=4) as pdw8,
            tc.tile_pool(name="pdw", bufs=2 * NI + 2) as pdw,
            tc.tile_pool(name="pht", bufs=NI) as pht,
            tc.tile_pool(name="ptmp", bufs=1) as ptmp,
            tc.tile_pool(name="pout", bufs=4) as pout,
            tc.tile_pool(name="pprobe", bufs=2) as pprobe,
            tc.tile_pool(name="psg", bufs=1, space="PSUM") as psg,
            tc.tile_pool(name="psu", bufs=1, space="PSUM") as psu,
            tc.tile_pool(name="psd", bufs=2, space="PSUM") as psd,
        ):
            qs = [nc.gpsimd, nc.scalar, nc.sync]

            # x DMA in 1MB chunks round-robined over the queues so the
            # small down-phase output DMAs can interleave
            xstep = max(1, KO // 8)
            nxch = KO // xstep

            def load_x(b):
                x_sb = px.tile([128, KO, TB], BF16, tag="x", name="x_sb")
                for c in range(nxch):
                    qs[c % 3].dma_start(x_sb[:, c * xstep:(c + 1) * xstep, :],
                                        xq[b, :, c * xstep:(c + 1) * xstep, :])
                if share_x:
                    x2_sb = x_sb
                else:
                    x2_sb = px.tile([128, KO, TB], BF16, tag="x2", name="x2_sb")
                    for c in range(nxch):
                        qs[c % 3].dma_start(x2_sb[:, c * xstep:(c + 1) * xstep, :],
                                            xq2[b, :, c * xstep:(c + 1) * xstep, :])
                return x_sb, x2_sb

            # gate/up weights: int8 DMA + DVE dequant to bf16
            def load_w8(i):
                wg8_sb = pwi.tile([128, KO, 128], I8, tag="wg8", name="wg8_sb")
                nc.sync.dma_start(wg8_sb[:], wg8[i])
                wu8_sb = pwi.tile([128, KO, 128], I8, tag="wu8", name="wu8_sb")
                nc.sync.dma_start(wu8_sb[:], wu8[i])
                return wg8_sb, wu8_sb

            def conv_w(w8pair):
                wg8_sb, wu8_sb = w8pair
                wg16 = pw.tile([128, KO, 128], BF16, tag="wg16", name="wg16")
                nc.vector.tensor_copy(wg16[:], wg8_sb[:])
                wu16 = pw.tile([128, KO, 128], BF16, tag="wu16", name="wu16")
                nc.vector.tensor_copy(wu16[:], wu8_sb[:])
                return wg16, wu16

            # down_wT for one H pass: int8 DMA + dequant, NI tiles of
            # [128, HPW] that stay live for the whole pass
            def emit_wdt_pass(p):
                tiles = []
                for k in range(NI):
                    w8 = pdw8.tile([128, HPW], I8, tag="wdt8", name="w8")
                    qs[k % 2].dma_start(w8[:], wdt8[k][:, p * HPW:(p + 1) * HPW])
                    w16 = pdw.tile([128, HPW], BF16, tag="wdt16", name="w16")
                    nc.vector.tensor_copy(w16[:], w8[:])
                    tiles.append(w16)
                return tiles

            x_cur = load_x(0)
            w16_cur = conv_w(load_w8(0))

            for b in range(NB):
                x_sb, x2_sb = x_cur

                ht_tiles = []
                for i in range(NI):
                    wg16, wu16 = w16_cur

                    # gate / up chains: both 512-halves share each weight tile
                    g_ps = [psg.tile([128, 512], F32, tag=f"g{h}", name=f"g{h}")
                            for h in range(2)]
                    for ko in range(KO):
                        wt = wg16[:, ko, :]
                        for h in range(2):
                            nc.tensor.matmul(g_ps[h][:], wt,
                                             x_sb[:, ko, h * 512:(h + 1) * 512],
                                             start=(ko == 0), stop=(ko == KO - 1))
                    u_ps = [psu.tile([128, 512], F32, tag=f"u{h}", name=f"u{h}")
                            for h in range(2)]
                    for ko in range(KO):
                        wt = wu16[:, ko, :]
                        for h in range(2):
                            nc.tensor.matmul(u_ps[h][:], wt,
                                             x2_sb[:, ko, h * 512:(h + 1) * 512],
                                             start=(ko == 0), stop=(ko == KO - 1))

                    # prefetch + dequant the next weight pair ahead of the
                    # post-processing ops in the DVE queue
                    if not (b == NB - 1 and i == NI - 1):
                        w16_cur = conv_w(load_w8((i + 1) % NI))
                    if i == NI - 2:
                        wdp_cur = emit_wdt_pass(0)

                    # hidden = silu(f16(g*gsc)) * (u*usc/dis), round+clip int8
                    ht_i = pht.tile([128, TB], BF16, tag="ht", name="ht_i")
                    for h in range(2):
                        sl = slice(h * 512, (h + 1) * 512)
                        t16 = ptmp.tile([128, 512], F16, tag=f"t16{h}", name="t16")
                        nc.scalar.activation(t16[:], g_ps[h][:], ACT.Copy, scale=gsc)
                        s16 = ptmp.tile([128, 512], F16, tag=f"s16{h}", name="s16")
                        nc.scalar.activation(s16[:], t16[:], ACT.Sigmoid)
                        sl16 = ptmp.tile([128, 512], F16, tag=f"sl16{h}", name="sl16")
                        nc.vector.tensor_tensor(sl16[:], t16[:], s16[:], ALU.mult)
                        h32 = ptmp.tile([128, 512], F32, tag=f"h32{h}", name="h32")
                        nc.vector.scalar_tensor_tensor(h32[:], u_ps[h][:],
                                                       usc_over_dis,
                                                       sl16[:], ALU.mult, ALU.mult)
                        # clamp to (-128.49, 127.49) pre-round: keeps magic-add
                        # in range and matches round-then-clip on boundaries
                        c32 = ptmp.tile([128, 512], F32, tag=f"c32{h}", name="c32")
                        nc.vector.tensor_scalar(c32[:], h32[:], -128.49, 127.49,
                                                ALU.max, ALU.min)
                        nc.vector.tensor_scalar(ht_i[:, sl], c32[:], MAGIC, MAGIC,
                                                ALU.add, ALU.subtract)
                    ht_tiles.append(ht_i)

                if b + 1 < NB:
                    x_cur = load_x(b + 1)

                # down proj, transposed: stationary = ht[k][:, 128-token
                # slice], moving = down_wT [128, 512 of H]; the moving chunks
                # per pass share each stationary -> LDWEIGHTS amortized
                for p in range(NHP):
                    tiles = wdp_cur
                    if p + 1 < NHP:
                        wdp_cur = emit_wdt_pass(p + 1)
                    for t in range(NT):
                        tsl = slice(t * 128, (t + 1) * 128)
                        d_ps = [psd.tile([128, HG], F32, tag=f"d{g}", name=f"d{g}")
                                for g in range(NHG)]
                        for k in range(NI):
                            st = ht_tiles[k][:, tsl]
                            for g in range(NHG):
                                csl = slice(g * HG, (g + 1) * HG)
                                nc.tensor.matmul(d_ps[g][:], st, tiles[k][:, csl],
                                                 start=(k == 0), stop=(k == NI - 1))
                        for g in range(NHG):
                            o_sb = pout.tile([128, HG], BF16, tag="o", name="o_sb")
                            nc.scalar.activation(o_sb[:], d_ps[g][:], ACT.Copy,
                                                 scale=dsc)
                            nc.sync.dma_start(
                                rs_in[b, t * 128:(t + 1) * 128,
                                      p * HPW + g * HG:p * HPW + (g + 1) * HG],
                                o_sb[:])

                # fence: probe-read one row of every 128-token region of
                # rs_in[b].  Each probe row overlaps all 8 writer DMAs of that
                # region, so its descriptor waits on their *completion*
                # (Tile's direct DMA->collective edge only orders dispatch —
                # HW-observed first-execution corruption without this).  The
                # DVE copy consumes the probe; the collective syncs on it.
                pw_ = H // 128
                pr = pprobe.tile([128, NT * pw_], BF16, tag="pr", name="pr")
                for t in range(NT):
                    nc.sync.dma_start(pr[:, t * pw_:(t + 1) * pw_],
                                      rs_in[b, t * 128:t * 128 + 1, :])
                pr2 = pprobe.tile([128, NT * pw_], BF16, tag="pr2", name="pr2")
                cp = nc.vector.tensor_copy(pr2[:], pr[:])
                cc = nc.gpsimd.collective_compute(
                    "ReduceScatter", mybir.AluOpType.add,
                    replica_groups=[list(range(NCORES))],
                    ins=[rs_in[b].opt()], outs=[rs_out[b].opt()])
                add_dep_helper(cc.ins, cp.ins, sync=True,
                               reason="cc waits rs_in probe round-trip")
                nc.gpsimd.dma_start(out[b], rs_out[b])


    nc.finalize()
    _prog_cache[key] = nc
    return nc


def _quant_tile_x(x: np.ndarray, scale: float) -> np.ndarray:
    """clip(round(x/scale)) -> tiled [NB, 128, KO, TB] bf16 (exact ints)."""
    q = np.clip(np.round(x / np.float32(scale)), -128, 127).astype(np.float32)
    return np.ascontiguousarray(
        q.reshape(NB, TB, KO, 128).transpose(0, 3, 2, 1)
    ).astype(ml_dtypes.bfloat16)


def _prepare_in_maps(x, gate_w, up_w, down_w, gis, uis, share_x):
    xq = _quant_tile_x(np.asarray(x, np.float32), gis)
    xq2 = None if share_x else _quant_tile_x(np.asarray(x, np.float32), uis)

    # zero-pad I (11008 -> 11264): padded gate/up rows give hidden=0 and the
    # padded down rows are 0, so the result is unchanged
    gw = np.zeros((IP, H), np.int8); gw[:I] = np.asarray(gate_w)
    uw = np.zeros((IP, H), np.int8); uw[:I] = np.asarray(up_w)
    dwt = np.zeros((IP, H), np.int8); dwt[:I] = np.asarray(down_w).T

    in_maps = []
    for c in range(NCORES):
        i0, i1 = c * ISH, (c + 1) * ISH
        wg_c = np.ascontiguousarray(
            gw[i0:i1].reshape(NI, 128, KO, 128).transpose(0, 3, 2, 1))
        wu_c = np.ascontiguousarray(
            uw[i0:i1].reshape(NI, 128, KO, 128).transpose(0, 3, 2, 1))
        wdt_c = np.ascontiguousarray(dwt[i0:i1].reshape(NI, 128, H))
        m = {"xq": xq, "wg8": wg_c, "wu8": wu_c, "wdt8": wdt_c}
        if not share_x:
            m["xq2"] = xq2
        in_maps.append(m)
    return in_maps


def kernel(x, gate_w, up_w, down_w,
           gate_in_scale, gate_w_scale,
           up_in_scale, up_w_scale,
           down_in_scale, down_w_scale):
    gis = float(gate_in_scale)
    uis = float(up_in_scale)
    dis = float(down_in_scale)
    gsc = float(np.float32(gis) * np.float32(gate_w_scale))
    usc = float(np.float32(uis) * np.float32(up_w_scale))
    dsc = float(np.float32(dis) * np.float32(down_w_scale))
    share_x = (np.float32(gis) == np.float32(uis))

    nc = _build_program(share_x, gsc, usc / dis, dsc)
    in_maps = _prepare_in_maps(x, gate_w, up_w, down_w, gis, uis, share_x)

    res = run_bass_kernel_spmd(nc, in_maps, list(range(NCORES)))

    final = np.empty((NB, NCORES, 128, H), np.float32)
    for c in range(NCORES):
        final[:, c] = np.asarray(res.results[c]["out"]).astype(np.float32)
    return final.reshape(T, H)
